# revision 39
# baseline (speedup 1.0000x reference)
"""ChebConv(K=3) x2 + BN GNN kernel for 8 Trainium2 NeuronCores.

Strategy:
  - Nodes dst-sharded across 8 cores (12500 each, padded to 12544 = 98*128).
  - ChebNet algebra refactored: out = x@(W0-W2) + L(x@W1) + L(L(x@(2W2)))
    with L = -D^-1/2 A D^-1/2 applied as: scale rows by dis on the way into
    the gather table, scale by -dis on the way out of the aggregation.
  - Each propagate: per dst-tile (128 dsts), gather source rows from a
    replicated table in HBM via dma_gather (int16 idx => 4 row-chunks of
    <=32768), build a 0/1 selection matrix on DVE (iota==dstloc), and
    aggregate with TensorE matmuls accumulating in PSUM.
  - Tables replicated across cores via AllGather after each half-step.
  - The kernel is DMA-descriptor-bound (1 descriptor per edge per propagate,
    ~40-65ns/descriptor on the 16 SDMA engines regardless of bytes), so
    table rows are kept at 256B DRAM pitch (stride must be a 256B multiple)
    while the gathered payload is shrunk:
      * layer-1 tables fp8 e4m3, gathered 128 feats = 128B/descriptor
        (rel err ~9e-3, within the 2e-2 budget);
      * layer-2 tables fp16, gathered 64 real feats = 128B/descriptor
        (exact - upper 64 cols were zero padding before);
    via dma_gather_raw, which emits InstDMAGatherAnt without the bass-level
    elem_size%256 assert (the non-transpose HBM ucode path supports any
    elem_size; only the stride is constrained).
  - prop1 is COMMUTED: it gathers tab0 = fp8(dis*x) (staged replicated on
    every core) instead of a dense-transformed table, because
    A(dis*x) @ 2W2 == A(dis*(x@2W2)). The aggregation matmul is operand-
    swapped (psum = g.T @ S = [feat, dst]) so the per-tile @2W2 follow-up
    matmul consumes it directly as lhsT after a psum->SBUF copy. prop1
    therefore starts at T=0 with no dense dependency and no AllGather.
  - AllGather outputs live in the Shared address window, and gathers from
    Shared DRAM run ~2x slower per descriptor than from normal DRAM; each
    table is bounced chunk-wise into a local DRAM tile (localize) before
    being gathered.
  - Aggregation matmuls keep UNIFORM operand dtypes (S built in the gather
    dtype): mixed f16xf8 matmuls run slow on PE and backpressure the
    gather pipeline.
  - Slot contents are sorted by source row within each (tile, chunk) group
    for ascending-address gathers.
"""
import sys
import time

for _p in ("/opt/trn_rl_repo",):
    if _p not in sys.path:
        sys.path.insert(0, _p)

import numpy as np

import concourse.bass as bass
import concourse.bacc as bacc
import concourse.mybir as mybir
import concourse.tile as tile
from concourse.masks import make_identity

N_CORES = 8
EPS = 1e-5
CHUNK = 32768  # dma_gather int16 index limit per chunk
NI_MAX_BATCHES = 8  # <=1024 idxs per dma_gather (HW ring limit)
SCRATCH = 16384  # SWDGE descriptor ring bytes/partition
WIDE_S = True  # one DVE selection-matrix build per gather call


def dma_gather_raw(nc, out_ap, in_ap, idxs_ap, num_idxs, elem_size,
                   stride_bytes, queue_num):
    """bass.dma_gather minus the elem_size_bytes%256 assert. The non-transpose
    HBM ucode path (q7 dma_gather.cpp gen_descs) supports arbitrary
    elem_size_bytes; only the row stride must be a 256B multiple."""
    eng = nc.gpsimd
    assert stride_bytes % 256 == 0
    _in_ap = eng.lower_ap_dma(in_ap, for_custom_bir_dma=True)
    _idxs_ap = eng.lower_ap(idxs_ap)
    _out_ap = eng.lower_ap(out_ap)
    return eng.add_instruction(
        mybir.InstDMAGatherAnt(
            name=nc.get_next_instruction_name(),
            ins=[*_in_ap, _idxs_ap,
                 eng.lower_val_access(eng.to_reg(num_idxs))],
            outs=[_out_ap],
            transpose=False,
            num_idxs=num_idxs,
            elem_size=elem_size,
            stride_bytes_256=stride_bytes // 256,
            gen_mode=0,
            single_packet=True,
            queue_num=queue_num,
            sbuf_tokens_per_rank=0,
            sbuf_free_dim_per_rank=0,
            sbuf_free_dim_pad_per_rank=0,
            sbuf_byte_offset=0,
        )
    )


def make_cfg(N, DIN, HID, OUT):
    SH = N // N_CORES
    assert SH * N_CORES == N
    TILES = (SH + 127) // 128
    SHP = TILES * 128
    TR = N_CORES * SHP  # table rows
    NCH = (TR + CHUNK - 1) // CHUNK
    return dict(N=N, DIN=DIN, HID=HID, OUT=OUT, SH=SH, SHP=SHP, TILES=TILES,
                TR=TR, NCH=NCH)


CFG = make_cfg(100000, 128, 128, 64)

# ---------------------------------------------------------------------------
# Host preprocessing
# ---------------------------------------------------------------------------


def preprocess_edges(edge_index, cfg):
    """Group edges by (dst shard, dst tile, src chunk). Stream order: for each
    tile-group (GSZ tiles), for each chunk, the group's tiles' padded slot
    runs contiguously -> gather calls of up to NI_MAX_BATCHES*128 idxs that
    span tiles. Builds a shared gather plan + per-core idx/dstloc streams."""
    N, SH, SHP, TILES, NCH = cfg["N"], cfg["SH"], cfg["SHP"], cfg["TILES"], cfg["NCH"]
    GSZ = 4
    src = edge_index[0].astype(np.int64)
    dst = edge_index[1].astype(np.int64)

    deg = np.bincount(src, minlength=N).astype(np.float64)
    dis = np.where(deg > 0, 1.0 / np.sqrt(np.maximum(deg, 1.0)), 0.0).astype(np.float32)

    shard = dst // SH
    tloc = (dst % SH) // 128
    rloc = (dst % SH) % 128
    rowof_all = (np.arange(N) // SH) * SHP + (np.arange(N) % SH)
    rowof = rowof_all[src]
    chunk = rowof // CHUNK

    key = (shard * TILES + tloc) * NCH + chunk
    order = np.argsort(key, kind="stable")
    grp_cnt = np.bincount(key, minlength=N_CORES * TILES * NCH)
    grp_start = np.zeros(N_CORES * TILES * NCH + 1, np.int64)
    np.cumsum(grp_cnt, out=grp_start[1:])
    nb = -(-grp_cnt.reshape(N_CORES, TILES, NCH) // 128)  # ceil
    nb_shared = nb.max(axis=0)  # [TILES, NCH]
    nb_shared[:, 0] = np.maximum(nb_shared[:, 0], 1)  # every tile >=1 batch

    B_total = int(nb_shared.sum())
    S_total = B_total * 128

    # ---- stream layout + gather plan ----
    # batches[gb] = (tile, chunk); slot offset of (t,ch) run start
    run_off = np.zeros((TILES, NCH), np.int64)
    batch_tile = np.zeros(B_total, np.int32)
    plan = []  # list of (chunk, slot0, nb_i, gb0)
    last_gb = np.zeros(TILES, np.int64)
    first_gb = np.full(TILES, -1, np.int64)
    pos = 0  # slot position
    gb = 0
    for g0 in range(0, TILES, GSZ):
        g1 = min(g0 + GSZ, TILES)
        for ch in range(NCH):
            run_nb = int(nb_shared[g0:g1, ch].sum())
            if run_nb == 0:
                continue
            # record run offsets per tile
            p = pos
            bstart = gb
            for t in range(g0, g1):
                run_off[t, ch] = p
                nbt = int(nb_shared[t, ch])
                for _ in range(nbt):
                    batch_tile[gb] = t
                    if first_gb[t] < 0:
                        first_gb[t] = gb
                    last_gb[t] = gb
                    gb += 1
                p += nbt * 128
            # gather calls covering this run
            sub = 0
            while sub < run_nb:
                nb_i = min(NI_MAX_BATCHES, run_nb - sub)
                plan.append((ch, pos + sub * 128, nb_i, bstart + sub))
                sub += nb_i
            pos += run_nb * 128
    assert pos == S_total and gb == B_total

    idx_stream = np.zeros((N_CORES, S_total), np.int16)
    dloc_stream = np.full((N_CORES, S_total), 255, np.int16)
    src_local = (rowof % CHUNK).astype(np.int16)
    s_sorted = src_local[order]
    r_sorted = rloc[order].astype(np.int16)

    for c in range(N_CORES):
        for t in range(TILES):
            for ch in range(NCH):
                g = (c * TILES + t) * NCH + ch
                n = grp_cnt[g]
                if n == 0:
                    continue
                a = grp_start[g]
                o = run_off[t, ch]
                # ascending src order within the group -> ascending HBM
                # addresses per gather call (DRAM locality)
                sub = np.argsort(s_sorted[a:a + n], kind="stable")
                idx_stream[c, o:o + n] = s_sorted[a:a + n][sub]
                dloc_stream[c, o:o + n] = r_sorted[a:a + n][sub]

    idx_w = idx_stream.reshape(N_CORES, S_total // 16, 16).transpose(0, 2, 1)
    idx_w = np.ascontiguousarray(np.tile(idx_w, (1, 8, 1)))
    dloc_t = dloc_stream.reshape(N_CORES, B_total, 128).transpose(0, 2, 1)
    dloc_t = np.ascontiguousarray(dloc_t).astype(np.float16)

    meta = dict(nb_shared=nb_shared, B_total=B_total, S_total=S_total,
                plan=tuple(plan), batch_tile=tuple(batch_tile.tolist()),
                first_gb=tuple(first_gb.tolist()), last_gb=tuple(last_gb.tolist()))
    return meta, dis, idx_w, dloc_t


def build_host_inputs(x, dis, weights, cfg):
    """Per-core input tensors (excluding idx/dloc)."""
    (W1, b1, W2, b2, g1, beta1, m1, v1, g2, beta2, m2, v2) = weights
    N, SH, SHP, TILES, TR = cfg["N"], cfg["SH"], cfg["SHP"], cfg["TILES"], cfg["TR"]
    DIN, HID, OUT = cfg["DIN"], cfg["HID"], cfg["OUT"]

    A1 = (g1 / np.sqrt(v1 + EPS)).astype(np.float32)
    C1 = (beta1 + (b1 - m1) * A1).astype(np.float32)
    A2 = (g2 / np.sqrt(v2 + EPS)).astype(np.float32)
    C2 = (beta2 + (b2 - m2) * A2).astype(np.float32)
    # BN scale folded into the weights (diag(A) commutes with L and with
    # the node-indexed sums); only the bias add remains on-chip
    Wcat1 = np.concatenate([(W1[0] - W1[2]) * A1, W1[1] * A1,
                            2.0 * W1[2] * A1], axis=1).astype(np.float16)
    Wcat2 = np.concatenate([(W2[0] - W2[2]) * A2, W2[1] * A2,
                            2.0 * W2[2] * A2], axis=1).astype(np.float16)
    AC1 = np.concatenate([np.tile(A1, (128, 1)), np.tile(C1, (128, 1))], axis=1)
    AC2 = np.concatenate([np.tile(A2, (128, 1)), np.tile(C2, (128, 1))], axis=1)

    import ml_dtypes
    tab0 = np.zeros((TR, 2 * DIN), ml_dtypes.float8_e4m3)
    xp = np.zeros((N_CORES, SHP, DIN), np.float32)
    disp = np.zeros((N_CORES, SHP), np.float32)
    for c in range(N_CORES):
        xs = x[c * SH:(c + 1) * SH]
        xp[c, :SH] = xs
        disp[c, :SH] = dis[c * SH:(c + 1) * SH]
        tab0[c * SHP:c * SHP + SH, :DIN] = (
            dis[c * SH:(c + 1) * SH, None] * xs).astype(ml_dtypes.float8_e4m3)

    in_maps = []
    for c in range(N_CORES):
        d = disp[c].reshape(TILES, 128).T  # [128, TILES]
        in_maps.append({
            "xT": np.ascontiguousarray(xp[c].T).astype(np.float16),
            "tab0": tab0,
            "disP": np.ascontiguousarray(d),
            "disN": np.ascontiguousarray(-d),
            "dis2N": np.ascontiguousarray(-(d.astype(np.float64) ** 2)).astype(np.float32),
            "Wcat1": Wcat1,
            "Wcat2": Wcat2,
            "AC1": AC1.astype(np.float32),
            "AC2": AC2.astype(np.float32),
        })
    return in_maps


# ---------------------------------------------------------------------------
# Bass program
# ---------------------------------------------------------------------------


def build_program(cfg, meta, repeat=1, parts="all", single=False, do_compile=True):
    dt = mybir.dt
    f16, f32, i16 = dt.float16, dt.float32, dt.int16
    f8 = dt.float8e4
    SHP, TILES, TR, NCH = cfg["SHP"], cfg["TILES"], cfg["TR"], cfg["NCH"]
    DIN, HID, OUT = cfg["DIN"], cfg["HID"], cfg["OUT"]
    nb_shared = meta["nb_shared"]
    B_total, S_total = meta["B_total"], meta["S_total"]

    nc = bacc.Bacc("TRN2", target_bir_lowering=False, debug=False,
                   num_devices=(1 if single else N_CORES), num_swdge_queues=4,
                   dynamic_dma_scratch_size=SCRATCH)

    xT_d = nc.dram_tensor("xT", [128, SHP], f16, kind="ExternalInput")
    tab0_d = nc.dram_tensor("tab0", [TR, 2 * DIN], f8, kind="ExternalInput")
    disP_d = nc.dram_tensor("disP", [128, TILES], f32, kind="ExternalInput")
    disN_d = nc.dram_tensor("disN", [128, TILES], f32, kind="ExternalInput")
    dis2N_d = nc.dram_tensor("dis2N", [128, TILES], f32, kind="ExternalInput")
    W1_d = nc.dram_tensor("Wcat1", [DIN, 3 * HID], f16, kind="ExternalInput")
    W2_d = nc.dram_tensor("Wcat2", [HID, 3 * OUT], f16, kind="ExternalInput")
    AC1_d = nc.dram_tensor("AC1", [128, 2 * HID], f32, kind="ExternalInput")
    AC2_d = nc.dram_tensor("AC2", [128, 2 * OUT], f32, kind="ExternalInput")
    idx_d = nc.dram_tensor("idxs", [128, S_total // 16], i16, kind="ExternalInput")
    dloc_d = nc.dram_tensor("dloc", [128, B_total], f16, kind="ExternalInput")
    out_d = nc.dram_tensor("out", [SHP, OUT], f32, kind="ExternalOutput")

    rg = [list(range(N_CORES))]

    with tile.TileContext(nc) as tc:
        import contextlib
        ctx = contextlib.ExitStack()
        with ctx:
            const_p = ctx.enter_context(tc.tile_pool(name="const", bufs=1))
            big_p = ctx.enter_context(tc.tile_pool(name="big", bufs=1))
            g16_p = ctx.enter_context(tc.tile_pool(name="g16", bufs=26))
            s_p = ctx.enter_context(tc.tile_pool(name="sel", bufs=10))
            ev_p = ctx.enter_context(tc.tile_pool(name="ev", bufs=3))
            ps_prop = ctx.enter_context(tc.tile_pool(name="psprop", bufs=5, space="PSUM"))
            ps_dense = ctx.enter_context(tc.tile_pool(name="psdense", bufs=1, space="PSUM"))
            ps_tr = ctx.enter_context(tc.tile_pool(name="pstr", bufs=1, space="PSUM"))
            ps_ev = ctx.enter_context(tc.tile_pool(name="psev", bufs=1, space="PSUM"))
            dram_p = ctx.enter_context(tc.tile_pool(name="dram", bufs=1, space="DRAM"))

            # ---- constants ----
            iota16_sb = const_p.tile([128, 128], f16)
            nc.gpsimd.iota(iota16_sb[:], pattern=[[1, 128]], base=0,
                           channel_multiplier=0,
                           allow_small_or_imprecise_dtypes=True)
            ident = const_p.tile([128, 128], f16)
            make_identity(nc, ident[:])
            W1_sb = const_p.tile([DIN, 3 * HID], f16)
            nc.sync.dma_start(out=W1_sb[:], in_=W1_d.ap())
            W2_sb = const_p.tile([HID, 3 * OUT], f16)
            nc.sync.dma_start(out=W2_sb[:], in_=W2_d.ap())
            AC1_sb = const_p.tile([128, 2 * HID], f32)
            nc.sync.dma_start(out=AC1_sb[:], in_=AC1_d.ap())
            AC2_sb = const_p.tile([128, 2 * OUT], f32)
            nc.sync.dma_start(out=AC2_sb[:], in_=AC2_d.ap())
            disP_sb = const_p.tile([128, TILES], f32)
            nc.sync.dma_start(out=disP_sb[:], in_=disP_d.ap())
            disN_sb = const_p.tile([128, TILES], f32)
            nc.sync.dma_start(out=disN_sb[:], in_=disN_d.ap())
            dis2N_sb = const_p.tile([128, TILES], f32)
            nc.sync.dma_start(out=dis2N_sb[:], in_=dis2N_d.ap())
            dloc16_sb = const_p.tile([128, B_total], f16)
            nc.sync.dma_start(out=dloc16_sb[:], in_=dloc_d.ap())
            idx_sb = const_p.tile([128, S_total // 16], i16)
            nc.sync.dma_start(out=idx_sb[:], in_=idx_d.ap())

            # ---- big resident arrays ----
            xT_sb = big_p.tile([128, SHP], f16, tag="xT")
            nc.sync.dma_start(out=xT_sb[:], in_=xT_d.ap())

            gq = [0]  # rotating gather queue
            rep_i = [0]

            def run_body():
                ri = rep_i[0]
                rep_i[0] += 1
                # ---- DRAM bounce + tables (fresh per repeat; Shared AG outs) ----
                za_sb = big_p.tile([128, TILES * HID], f16, tag="za")
                zb_sb = big_p.tile([128, TILES * HID], f16, tag="zb")
                b2_t = dram_p.tile([SHP, 2 * HID], f8, tag=f"b2_{ri}")
                t2_t = dram_p.tile([TR, 2 * HID], f8, addr_space="Shared", tag=f"t2_{ri}")
                b3_t = dram_p.tile([SHP, HID], f16, tag=f"b3_{ri}")
                t3_t = dram_p.tile([TR, HID], f16, addr_space="Shared", tag=f"t3_{ri}")
                b4_t = dram_p.tile([SHP, HID], f16, tag=f"b4_{ri}")
                t4_t = dram_p.tile([TR, HID], f16, addr_space="Shared", tag=f"t4_{ri}")
                # gathering from the Shared-window AG outputs runs ~2x slower
                # per descriptor than from normal DRAM; bounce each table into
                # a local tile (chunk-wise so early-chunk gathers can start
                # while later chunks still copy)
                t2L = dram_p.tile([TR, 2 * HID], f8, tag=f"t2L_{ri}")
                t3L = dram_p.tile([TR, HID], f16, tag=f"t3L_{ri}")
                t4L = dram_p.tile([TR, HID], f16, tag=f"t4L_{ri}")

                def localize(dst, src):
                    for ch in range(NCH):
                        r0 = ch * CHUNK
                        r1 = min((ch + 1) * CHUNK, TR)
                        nc.sync.dma_start(out=dst[r0:r1, :], in_=src[r0:r1, :])

                def dense(lhs_sb, W_sb, F, za_dst, zb_dst, bounce, ev_dtype,
                          skip_c=False):
                    """z = lhs.T @ [Wa|Wb|Wc]; za kept, zb=dis*z_b kept, z_c=dis*z_c -> bounce."""
                    nj = 2 if skip_c else 3
                    for t in range(TILES):
                        lhsT = lhs_sb[:, t * 128:(t + 1) * 128]
                        ps = ps_dense.tile([128, 3 * F], f32)
                        for j in range(nj):
                            nc.tensor.matmul(ps[:, j * F:(j + 1) * F], lhsT,
                                             W_sb[:, j * F:(j + 1) * F],
                                             start=True, stop=True)
                        nc.vector.tensor_copy(za_dst[:, t * F:(t + 1) * F], ps[:, 0:F])
                        nc.vector.tensor_scalar(zb_dst[:, t * F:(t + 1) * F],
                                                ps[:, F:2 * F], disP_sb[:, t:t + 1],
                                                None, mybir.AluOpType.mult)
                        if skip_c:
                            continue
                        zc = ev_p.tile([128, 3 * OUT if F == OUT else F], ev_dtype, tag="zc")
                        nc.vector.tensor_scalar(zc[:, :F], ps[:, 2 * F:3 * F],
                                                disP_sb[:, t:t + 1], None,
                                                mybir.AluOpType.mult)
                        nc.sync.dma_start(out=bounce[t * 128:(t + 1) * 128, 0:F],
                                          in_=zc[:, :F])

                em_g = parts in ("all", "gather", "gs")
                em_s = parts in ("all", "gs", "nogather")
                em_m = parts in ("all", "nogather")
                plan = meta["plan"]
                batch_tile = meta["batch_tile"]
                first_gb = meta["first_gb"]
                last_gb = meta["last_gb"]

                def propagate(table, F, gdt, g_pool, evac, swap=False,
                              table_early=None, n_early=0):
                    """y[dst] = sum_e table[src_e]; evac(t, psum) consumes PSUM.

                    swap=True computes the TRANSPOSED aggregate psum = g.T @ S
                    ([feat, dst] instead of [dst, feat]) so the evac can feed it
                    straight into a follow-up matmul as lhsT. S is built in the
                    gather dtype so the matmul operands match."""
                    psums = {}
                    s_dt = gdt  # uniform-dtype matmuls; mixed f16xf8 MMs stall PE
                    for ci, (ch, slot0, nb_i, gb0) in enumerate(plan):
                        # while the localize copies drain, the first calls read
                        # the Shared AG output directly (slower but not idle)
                        tab = table_early if (table_early is not None
                                              and ci < n_early) else table
                        rows0 = ch * CHUNK
                        rows1 = min((ch + 1) * CHUNK, TR)
                        ni = nb_i * 128
                        col0 = slot0 // 16
                        g = g_pool.tile([128, NI_MAX_BATCHES, F], gdt, tag="g")
                        if em_g:
                            ebytes = F * (1 if gdt == f8 else 2)
                            if ebytes % 256 == 0:
                                nc.gpsimd.dma_gather(
                                    out_ap=g[:, :nb_i, :], in_ap=tab[rows0:rows1, :],
                                    idxs_ap=idx_sb[:, col0:col0 + ni // 16], num_idxs=ni,
                                    num_idxs_reg=ni, elem_size=F,
                                    queue_num=gq[0] % 4)
                            else:
                                # narrow rows (e.g. 64 f16 = 128B) at 256B pitch
                                dma_gather_raw(
                                    nc, g[:, :nb_i, :], tab[rows0:rows1, :],
                                    idx_sb[:, col0:col0 + ni // 16], ni, F,
                                    256, gq[0] % 4)
                            gq[0] += 1
                        if em_s and WIDE_S:
                            Sw = s_p.tile([128, NI_MAX_BATCHES, 128], s_dt, tag="S")
                            nc.vector.tensor_tensor(
                                out=Sw[:, :nb_i, :],
                                in0=iota16_sb[:].unsqueeze(1).broadcast_to([128, nb_i, 128]),
                                in1=dloc16_sb[:, gb0:gb0 + nb_i].unsqueeze(2).broadcast_to([128, nb_i, 128]),
                                op=mybir.AluOpType.is_equal)
                        for b in range(nb_i):
                            gb = gb0 + b
                            t = batch_tile[gb]
                            if em_m:
                                lhs = Sw[:, b, :]
                                if t not in psums:
                                    psums[t] = ps_prop.tile([128, F], f32, tag="pp", name=f"pp_{t}")
                                if swap:
                                    nc.tensor.matmul(psums[t][:], g[:, b, :], lhs,
                                                     start=(gb == first_gb[t]),
                                                     stop=(gb == last_gb[t]))
                                else:
                                    nc.tensor.matmul(psums[t][:], lhs, g[:, b, :],
                                                     start=(gb == first_gb[t]),
                                                     stop=(gb == last_gb[t]))
                                if gb == last_gb[t]:
                                    evac(t, psums.pop(t))

                if not em_m:
                    # timing-only: 4 propagates' gather/S traffic vs input table
                    for _ in range(4):
                        propagate(tab0_d.ap(), HID, f8, g16_p, None)
                    return

                # ================= layer 1 =================
                # prop1 gathers tab0 = fp8(dis*x) directly (already replicated
                # at staging): A(dis*x) @ 2W2 == A(dis*(x@2W2)), so the dense
                # transform moves AFTER the aggregation -> no dense dependency
                # and no AllGather before the first propagate. The swapped
                # aggregation leaves psum = p1'.T = [xfeat, dst], which feeds
                # the per-tile @2W2 matmul as lhsT after a psum->SBUF copy.
                dense(xT_sb, W1_sb, HID, za_sb, zb_sb, None, f8, skip_c=True)

                def evac_p1(t, psT):
                    cT = ev_p.tile([128, HID], f16, tag="cT")
                    nc.vector.tensor_copy(cT[:], psT[:])
                    ps2 = ps_ev.tile([128, HID], f32)
                    nc.tensor.matmul(ps2[:], cT[:], W1_sb[:, 2 * HID:3 * HID],
                                     start=True, stop=True)
                    tmp = ev_p.tile([128, HID], f16, tag="tmp16")
                    nc.vector.tensor_scalar(tmp[:], ps2[:], dis2N_sb[:, t:t + 1], None,
                                            mybir.AluOpType.mult)
                    v = ev_p.tile([128, HID], f8, tag="v8")
                    nc.vector.tensor_tensor(out=v[:], in0=tmp[:],
                                            in1=zb_sb[:, t * HID:(t + 1) * HID],
                                            op=mybir.AluOpType.add)
                    nc.sync.dma_start(out=b2_t[t * 128:(t + 1) * 128, 0:HID], in_=v[:])

                propagate(tab0_d.ap(), HID, f8, g16_p, evac_p1, swap=True)

                if single:
                    nc.sync.dma_start(out=t2_t[0:SHP, :], in_=b2_t[:, :])
                else:
                    nc.gpsimd.collective_compute(
                        "AllGather", mybir.AluOpType.bypass,
                        ins=[b2_t[:, :]], outs=[t2_t[:, :]], replica_groups=rg)
                localize(t2L, t2_t)

                hT_sb = big_p.tile([128, SHP], f16, tag=("xT" if repeat == 1 else "hT"))  # reuse xT slot

                def evac_p2(t, ps):
                    s1 = ev_p.tile([128, HID], f32, tag="s1")
                    nc.vector.tensor_scalar(s1[:], ps[:], disN_sb[:, t:t + 1], None,
                                            mybir.AluOpType.mult)
                    s2 = ev_p.tile([128, HID], f32, tag="s2")
                    nc.vector.tensor_tensor(out=s2[:], in0=s1[:],
                                            in1=za_sb[:, t * HID:(t + 1) * HID],
                                            op=mybir.AluOpType.add)
                    s4 = ev_p.tile([128, HID], f32, tag="s2")
                    nc.vector.tensor_tensor(out=s4[:], in0=s2[:], in1=AC1_sb[:, HID:],
                                            op=mybir.AluOpType.add)
                    h = ev_p.tile([128, HID], f16, tag="h")
                    nc.vector.tensor_scalar(h[:], s4[:], 0.0, None,
                                            mybir.AluOpType.max)
                    pst = ps_tr.tile([128, 128], f16)
                    nc.tensor.transpose(out=pst[:], in_=h[:], identity=ident[:])
                    nc.vector.tensor_copy(hT_sb[:, t * 128:(t + 1) * 128], pst[:])

                propagate(t2L, HID, f8, g16_p, evac_p2)

                # ================= layer 2 =================
                za2_sb = big_p.tile([128, TILES * OUT], f32, tag="za")
                zb2_sb = big_p.tile([128, TILES * OUT], f32, tag="zb")

                def dense2():
                    for t in range(TILES):
                        lhsT = hT_sb[:, t * 128:(t + 1) * 128]
                        ps = ps_dense.tile([128, 3 * OUT], f32)
                        for j in range(3):
                            nc.tensor.matmul(ps[:, j * OUT:(j + 1) * OUT], lhsT,
                                             W2_sb[:, j * OUT:(j + 1) * OUT],
                                             start=True, stop=True)
                        nc.vector.tensor_copy(za2_sb[:, t * OUT:(t + 1) * OUT], ps[:, 0:OUT])
                        nc.vector.tensor_scalar(zb2_sb[:, t * OUT:(t + 1) * OUT],
                                                ps[:, OUT:2 * OUT], disP_sb[:, t:t + 1],
                                                None, mybir.AluOpType.mult)
                        zc = ev_p.tile([128, HID], f16, tag="zc")
                        nc.vector.tensor_scalar(zc[:, :OUT], ps[:, 2 * OUT:3 * OUT],
                                                disP_sb[:, t:t + 1], None,
                                                mybir.AluOpType.mult)
                        nc.vector.memset(zc[:, OUT:], 0.0)
                        nc.sync.dma_start(out=b3_t[t * 128:(t + 1) * 128, :],
                                          in_=zc[:, :])

                dense2()

                if single:
                    nc.sync.dma_start(out=t3_t[0:SHP, :], in_=b3_t[:, :])
                else:
                    nc.gpsimd.collective_compute(
                        "AllGather", mybir.AluOpType.bypass,
                        ins=[b3_t[:, :]], outs=[t3_t[:, :]], replica_groups=rg)
                localize(t3L, t3_t)

                def evac_p3(t, ps):
                    tmp = ev_p.tile([128, OUT], f32, tag="tmp32")
                    nc.vector.tensor_scalar(tmp[:], ps[:], dis2N_sb[:, t:t + 1], None,
                                            mybir.AluOpType.mult)
                    v = ev_p.tile([128, HID], f16, tag="v16b")
                    nc.vector.tensor_tensor(out=v[:, :OUT], in0=tmp[:],
                                            in1=zb2_sb[:, t * OUT:(t + 1) * OUT],
                                            op=mybir.AluOpType.add)
                    nc.vector.memset(v[:, OUT:], 0.0)
                    nc.sync.dma_start(out=b4_t[t * 128:(t + 1) * 128, :], in_=v[:, :])

                propagate(t3L, OUT, f16, g16_p, evac_p3)

                if single:
                    nc.sync.dma_start(out=t4_t[0:SHP, :], in_=b4_t[:, :])
                else:
                    nc.gpsimd.collective_compute(
                        "AllGather", mybir.AluOpType.bypass,
                        ins=[b4_t[:, :]], outs=[t4_t[:, :]], replica_groups=rg)
                localize(t4L, t4_t)

                def evac_p4(t, ps):
                    o1 = ev_p.tile([128, OUT], f32, tag="o1")
                    nc.vector.tensor_scalar(o1[:], ps[:], disN_sb[:, t:t + 1], None,
                                            mybir.AluOpType.mult)
                    o2 = ev_p.tile([128, OUT], f32, tag="o2")
                    nc.vector.tensor_tensor(out=o2[:], in0=o1[:],
                                            in1=za2_sb[:, t * OUT:(t + 1) * OUT],
                                            op=mybir.AluOpType.add)
                    o4 = ev_p.tile([128, OUT], f32, tag="o2")
                    nc.vector.tensor_tensor(out=o4[:], in0=o2[:], in1=AC2_sb[:, OUT:],
                                            op=mybir.AluOpType.add)
                    nc.sync.dma_start(out=out_d.ap()[t * 128:(t + 1) * 128, :], in_=o4[:])

                propagate(t4L, OUT, f16, g16_p, evac_p4)

            for _rep in range(repeat):
                run_body()


    if do_compile:
        nc.compile()
    return nc


# ---------------------------------------------------------------------------
# SPMD runner (axon / PJRT path), kept warm across calls
# ---------------------------------------------------------------------------


class SpmdRunner:
    def __init__(self, nc, n_cores=N_CORES):
        import jax
        from jax.sharding import Mesh, PartitionSpec, NamedSharding
        from jax.experimental.shard_map import shard_map
        from concourse.bass2jax import (_bass_exec_p, partition_id_tensor,
                                        install_neuronx_cc_hook)
        install_neuronx_cc_hook()
        self.jax = jax
        self.n_cores = n_cores
        partition_name = nc.partition_id_tensor.name if nc.partition_id_tensor else None
        in_names, out_names, out_avals, zero_outs = [], [], [], []
        for alloc in nc.m.functions[0].allocations:
            if not isinstance(alloc, mybir.MemoryLocationSet):
                continue
            name = alloc.memorylocations[0].name
            if alloc.kind == "ExternalInput":
                if name != partition_name:
                    in_names.append(name)
            elif alloc.kind == "ExternalOutput":
                out_names.append(name)
                shape = tuple(alloc.tensor_shape)
                dtype = mybir.dt.np(alloc.dtype)
                out_avals.append(jax.core.ShapedArray(shape, dtype))
                zero_outs.append(np.zeros(shape, dtype))
        self.in_names, self.out_names = in_names, out_names
        self.out_avals, self.zero_outs = out_avals, zero_outs
        all_in_names = list(in_names) + list(out_names)
        if partition_name is not None:
            all_in_names.append(partition_name)

        def _body(*args):
            operands = list(args)
            if partition_name is not None:
                operands.append(partition_id_tensor())
            outs = _bass_exec_p.bind(
                *operands,
                out_avals=tuple(out_avals),
                in_names=tuple(all_in_names),
                out_names=tuple(out_names),
                lowering_input_output_aliases=(),
                sim_require_finite=True,
                sim_require_nnan=True,
                nc=nc,
            )
            return tuple(outs)

        devices = jax.devices()[:n_cores]
        self.mesh = Mesh(np.asarray(devices), ("core",))
        spec = PartitionSpec("core")
        self.sharding = NamedSharding(self.mesh, spec)
        in_specs = (spec,) * (len(in_names) + len(out_names))
        out_specs = (spec,) * len(out_names)
        self.fn = jax.jit(
            shard_map(_body, mesh=self.mesh, in_specs=in_specs,
                      out_specs=out_specs, check_rep=False),
            keep_unused=True,
        )

    def stage(self, in_maps):
        concat_in = [
            np.concatenate([np.asarray(in_maps[c][n]) for c in range(self.n_cores)], axis=0)
            for n in self.in_names
        ]
        concat_zeros = [
            np.zeros((self.n_cores * z.shape[0], *z.shape[1:]), z.dtype)
            for z in self.zero_outs
        ]
        dev = [self.jax.device_put(a, self.sharding) for a in concat_in + concat_zeros]
        self.jax.block_until_ready(dev)
        return dev

    def run(self, staged):
        out = self.fn(*staged)
        self.jax.block_until_ready(out)
        return out

    def unpack(self, out_arrs):
        res = []
        for c in range(self.n_cores):
            d = {}
            for i, n in enumerate(self.out_names):
                d[n] = np.asarray(out_arrs[i]).reshape(
                    self.n_cores, *self.out_avals[i].shape)[c]
            res.append(d)
        return res


_CACHE = {}


def _get_runner(cfg, meta):
    key = (tuple(sorted(cfg.items())), meta["nb_shared"].tobytes())
    if key not in _CACHE:
        nc = build_program(cfg, meta)
        _CACHE[key] = SpmdRunner(nc)
    return _CACHE[key]


def run_model(x, edge_index, weights, cfg):
    meta, dis, idx_w, dloc_t = preprocess_edges(edge_index, cfg)
    in_maps = build_host_inputs(x, dis, weights, cfg)
    for c in range(N_CORES):
        in_maps[c]["idxs"] = idx_w[c]
        in_maps[c]["dloc"] = dloc_t[c]
    r = _get_runner(cfg, meta)
    staged = r.stage(in_maps)
    res = r.unpack(r.run(staged))
    N, SH, OUT = cfg["N"], cfg["SH"], cfg["OUT"]
    out = np.empty((N, OUT), np.float32)
    for c in range(N_CORES):
        out[c * SH:(c + 1) * SH] = res[c]["out"][:SH]
    return out


def kernel(x, edge_index, W1, b1, W2, b2, g1, beta1, m1, v1, g2, beta2, m2, v2):
    x = np.asarray(x, np.float32)
    edge_index = np.asarray(edge_index)
    weights = tuple(np.asarray(w, np.float32) for w in
                    (W1, b1, W2, b2, g1, beta1, m1, v1, g2, beta2, m2, v2))
    return run_model(x, edge_index, weights, CFG)



# revision 42
# speedup vs baseline: 1.1310x; 1.1310x over previous
"""ChebConv(K=3) x2 + BN GNN kernel for 8 Trainium2 NeuronCores.

Strategy:
  - Nodes dst-sharded across 8 cores (12500 each, padded to 12544 = 98*128).
  - ChebNet algebra refactored: out = x@(W0-W2) + L(x@W1) + L(L(x@(2W2)))
    with L = -D^-1/2 A D^-1/2 applied as: scale rows by dis on the way into
    the gather table, scale by -dis on the way out of the aggregation.
  - Each propagate: per dst-tile (128 dsts), gather source rows from a
    replicated table in HBM via dma_gather (int16 idx => 4 row-chunks of
    <=32768), build a 0/1 selection matrix on DVE (iota==dstloc), and
    aggregate with TensorE matmuls accumulating in PSUM.
  - Tables replicated across cores via AllGather after each half-step.
  - The kernel is DMA-descriptor-bound (1 descriptor per edge per propagate,
    ~40-65ns/descriptor on the 16 SDMA engines regardless of bytes), so
    table rows are kept at 256B DRAM pitch (stride must be a 256B multiple)
    while the gathered payload is shrunk:
      * layer-1 tables fp8 e4m3, gathered 128 feats = 128B/descriptor
        (rel err ~9e-3, within the 2e-2 budget);
      * layer-2 tables fp16, gathered 64 real feats = 128B/descriptor
        (exact - upper 64 cols were zero padding before);
    via dma_gather_raw, which emits InstDMAGatherAnt without the bass-level
    elem_size%256 assert (the non-transpose HBM ucode path supports any
    elem_size; only the stride is constrained).
  - prop1 is COMMUTED: it gathers tab0 = fp8(dis*x) (staged replicated on
    every core) instead of a dense-transformed table, because
    A(dis*x) @ 2W2 == A(dis*(x@2W2)). The aggregation matmul is operand-
    swapped (psum = g.T @ S = [feat, dst]) so the per-tile @2W2 follow-up
    matmul consumes it directly as lhsT after a psum->SBUF copy. prop1
    therefore starts at T=0 with no dense dependency and no AllGather.
  - AllGather outputs live in the Shared address window, and gathers from
    Shared DRAM run ~2x slower per descriptor than from normal DRAM; each
    table is bounced chunk-wise into a local DRAM tile (localize) before
    being gathered.
  - Aggregation matmuls keep UNIFORM operand dtypes (S built in the gather
    dtype): mixed f16xf8 matmuls run slow on PE and backpressure the
    gather pipeline.
  - The BN scale vectors A1/A2 are folded into the Wcat blocks host-side
    (diag(A) commutes with L and the node sums); only the bias add remains
    in the evacuations. This also improves fp8 table quantization slightly.
  - Slot contents are sorted by source row within each (tile, chunk) group
    for ascending-address gathers.
"""
import sys
import time

for _p in ("/opt/trn_rl_repo",):
    if _p not in sys.path:
        sys.path.insert(0, _p)

import numpy as np

import concourse.bass as bass
import concourse.bacc as bacc
import concourse.mybir as mybir
import concourse.tile as tile
from concourse.masks import make_identity

N_CORES = 8
EPS = 1e-5
CHUNK = 32768  # dma_gather int16 index limit per chunk
NI_MAX_BATCHES = 8  # <=1024 idxs per dma_gather (HW ring limit)
SCRATCH = 16384  # SWDGE descriptor ring bytes/partition
WIDE_S = True  # one DVE selection-matrix build per gather call


def dma_gather_raw(nc, out_ap, in_ap, idxs_ap, num_idxs, elem_size,
                   stride_bytes, queue_num):
    """bass.dma_gather minus the elem_size_bytes%256 assert. The non-transpose
    HBM ucode path (q7 dma_gather.cpp gen_descs) supports arbitrary
    elem_size_bytes; only the row stride must be a 256B multiple."""
    eng = nc.gpsimd
    assert stride_bytes % 256 == 0
    _in_ap = eng.lower_ap_dma(in_ap, for_custom_bir_dma=True)
    _idxs_ap = eng.lower_ap(idxs_ap)
    _out_ap = eng.lower_ap(out_ap)
    return eng.add_instruction(
        mybir.InstDMAGatherAnt(
            name=nc.get_next_instruction_name(),
            ins=[*_in_ap, _idxs_ap,
                 eng.lower_val_access(eng.to_reg(num_idxs))],
            outs=[_out_ap],
            transpose=False,
            num_idxs=num_idxs,
            elem_size=elem_size,
            stride_bytes_256=stride_bytes // 256,
            gen_mode=0,
            single_packet=True,
            queue_num=queue_num,
            sbuf_tokens_per_rank=0,
            sbuf_free_dim_per_rank=0,
            sbuf_free_dim_pad_per_rank=0,
            sbuf_byte_offset=0,
        )
    )


def make_cfg(N, DIN, HID, OUT):
    SH = N // N_CORES
    assert SH * N_CORES == N
    TILES = (SH + 127) // 128
    SHP = TILES * 128
    TR = N_CORES * SHP  # table rows
    NCH = (TR + CHUNK - 1) // CHUNK
    return dict(N=N, DIN=DIN, HID=HID, OUT=OUT, SH=SH, SHP=SHP, TILES=TILES,
                TR=TR, NCH=NCH)


CFG = make_cfg(100000, 128, 128, 64)

# ---------------------------------------------------------------------------
# Host preprocessing
# ---------------------------------------------------------------------------


def preprocess_edges(edge_index, cfg):
    """Group edges by (dst shard, dst tile, src chunk). Stream order: for each
    tile-group (GSZ tiles), for each chunk, the group's tiles' padded slot
    runs contiguously -> gather calls of up to NI_MAX_BATCHES*128 idxs that
    span tiles. Builds a shared gather plan + per-core idx/dstloc streams."""
    N, SH, SHP, TILES, NCH = cfg["N"], cfg["SH"], cfg["SHP"], cfg["TILES"], cfg["NCH"]
    GSZ = 4
    src = edge_index[0].astype(np.int64)
    dst = edge_index[1].astype(np.int64)

    deg = np.bincount(src, minlength=N).astype(np.float64)
    dis = np.where(deg > 0, 1.0 / np.sqrt(np.maximum(deg, 1.0)), 0.0).astype(np.float32)

    shard = dst // SH
    tloc = (dst % SH) // 128
    rloc = (dst % SH) % 128
    rowof_all = (np.arange(N) // SH) * SHP + (np.arange(N) % SH)
    rowof = rowof_all[src]
    chunk = rowof // CHUNK

    key = (shard * TILES + tloc) * NCH + chunk
    order = np.argsort(key, kind="stable")
    grp_cnt = np.bincount(key, minlength=N_CORES * TILES * NCH)
    grp_start = np.zeros(N_CORES * TILES * NCH + 1, np.int64)
    np.cumsum(grp_cnt, out=grp_start[1:])
    nb = -(-grp_cnt.reshape(N_CORES, TILES, NCH) // 128)  # ceil
    nb_shared = nb.max(axis=0)  # [TILES, NCH]
    nb_shared[:, 0] = np.maximum(nb_shared[:, 0], 1)  # every tile >=1 batch

    B_total = int(nb_shared.sum())
    S_total = B_total * 128

    # ---- stream layout + gather plan ----
    # batches[gb] = (tile, chunk); slot offset of (t,ch) run start
    run_off = np.zeros((TILES, NCH), np.int64)
    batch_tile = np.zeros(B_total, np.int32)
    plan = []  # list of (chunk, slot0, nb_i, gb0)
    last_gb = np.zeros(TILES, np.int64)
    first_gb = np.full(TILES, -1, np.int64)
    pos = 0  # slot position
    gb = 0
    for g0 in range(0, TILES, GSZ):
        g1 = min(g0 + GSZ, TILES)
        for ch in range(NCH):
            run_nb = int(nb_shared[g0:g1, ch].sum())
            if run_nb == 0:
                continue
            # record run offsets per tile
            p = pos
            bstart = gb
            for t in range(g0, g1):
                run_off[t, ch] = p
                nbt = int(nb_shared[t, ch])
                for _ in range(nbt):
                    batch_tile[gb] = t
                    if first_gb[t] < 0:
                        first_gb[t] = gb
                    last_gb[t] = gb
                    gb += 1
                p += nbt * 128
            # gather calls covering this run
            sub = 0
            while sub < run_nb:
                nb_i = min(NI_MAX_BATCHES, run_nb - sub)
                plan.append((ch, pos + sub * 128, nb_i, bstart + sub))
                sub += nb_i
            pos += run_nb * 128
    assert pos == S_total and gb == B_total

    idx_stream = np.zeros((N_CORES, S_total), np.int16)
    dloc_stream = np.full((N_CORES, S_total), 255, np.int16)
    src_local = (rowof % CHUNK).astype(np.int16)
    s_sorted = src_local[order]
    r_sorted = rloc[order].astype(np.int16)

    for c in range(N_CORES):
        for t in range(TILES):
            for ch in range(NCH):
                g = (c * TILES + t) * NCH + ch
                n = grp_cnt[g]
                if n == 0:
                    continue
                a = grp_start[g]
                o = run_off[t, ch]
                # ascending src order within the group -> ascending HBM
                # addresses per gather call (DRAM locality)
                sub = np.argsort(s_sorted[a:a + n], kind="stable")
                idx_stream[c, o:o + n] = s_sorted[a:a + n][sub]
                dloc_stream[c, o:o + n] = r_sorted[a:a + n][sub]

    idx_w = idx_stream.reshape(N_CORES, S_total // 16, 16).transpose(0, 2, 1)
    idx_w = np.ascontiguousarray(np.tile(idx_w, (1, 8, 1)))
    dloc_t = dloc_stream.reshape(N_CORES, B_total, 128).transpose(0, 2, 1)
    dloc_t = np.ascontiguousarray(dloc_t).astype(np.float16)

    meta = dict(nb_shared=nb_shared, B_total=B_total, S_total=S_total,
                plan=tuple(plan), batch_tile=tuple(batch_tile.tolist()),
                first_gb=tuple(first_gb.tolist()), last_gb=tuple(last_gb.tolist()))
    return meta, dis, idx_w, dloc_t


def build_host_inputs(x, dis, weights, cfg):
    """Per-core input tensors (excluding idx/dloc)."""
    (W1, b1, W2, b2, g1, beta1, m1, v1, g2, beta2, m2, v2) = weights
    N, SH, SHP, TILES, TR = cfg["N"], cfg["SH"], cfg["SHP"], cfg["TILES"], cfg["TR"]
    DIN, HID, OUT = cfg["DIN"], cfg["HID"], cfg["OUT"]

    A1 = (g1 / np.sqrt(v1 + EPS)).astype(np.float32)
    C1 = (beta1 + (b1 - m1) * A1).astype(np.float32)
    A2 = (g2 / np.sqrt(v2 + EPS)).astype(np.float32)
    C2 = (beta2 + (b2 - m2) * A2).astype(np.float32)
    # BN scale folded into the weights (diag(A) commutes with L and with
    # the node-indexed sums); only the bias add remains on-chip
    Wcat1 = np.concatenate([(W1[0] - W1[2]) * A1, W1[1] * A1,
                            2.0 * W1[2] * A1], axis=1).astype(np.float16)
    Wcat2 = np.concatenate([(W2[0] - W2[2]) * A2, W2[1] * A2,
                            2.0 * W2[2] * A2], axis=1).astype(np.float16)
    AC1 = np.concatenate([np.tile(A1, (128, 1)), np.tile(C1, (128, 1))], axis=1)
    AC2 = np.concatenate([np.tile(A2, (128, 1)), np.tile(C2, (128, 1))], axis=1)

    import ml_dtypes
    tab0 = np.zeros((TR, 2 * DIN), ml_dtypes.float8_e4m3)
    xp = np.zeros((N_CORES, SHP, DIN), np.float32)
    disp = np.zeros((N_CORES, SHP), np.float32)
    for c in range(N_CORES):
        xs = x[c * SH:(c + 1) * SH]
        xp[c, :SH] = xs
        disp[c, :SH] = dis[c * SH:(c + 1) * SH]
        tab0[c * SHP:c * SHP + SH, :DIN] = (
            dis[c * SH:(c + 1) * SH, None] * xs).astype(ml_dtypes.float8_e4m3)

    in_maps = []
    for c in range(N_CORES):
        d = disp[c].reshape(TILES, 128).T  # [128, TILES]
        in_maps.append({
            "xT": np.ascontiguousarray(xp[c].T).astype(np.float16),
            "tab0": tab0,
            "disP": np.ascontiguousarray(d),
            "disN": np.ascontiguousarray(-d),
            "dis2N": np.ascontiguousarray(-(d.astype(np.float64) ** 2)).astype(np.float32),
            "Wcat1": Wcat1,
            "Wcat2": Wcat2,
            "AC1": AC1.astype(np.float32),
            "AC2": AC2.astype(np.float32),
        })
    return in_maps


# ---------------------------------------------------------------------------
# Bass program
# ---------------------------------------------------------------------------


def build_program(cfg, meta, repeat=1, parts="all", single=False, do_compile=True):
    dt = mybir.dt
    f16, f32, i16 = dt.float16, dt.float32, dt.int16
    f8 = dt.float8e4
    SHP, TILES, TR, NCH = cfg["SHP"], cfg["TILES"], cfg["TR"], cfg["NCH"]
    DIN, HID, OUT = cfg["DIN"], cfg["HID"], cfg["OUT"]
    nb_shared = meta["nb_shared"]
    B_total, S_total = meta["B_total"], meta["S_total"]

    nc = bacc.Bacc("TRN2", target_bir_lowering=False, debug=False,
                   num_devices=(1 if single else N_CORES), num_swdge_queues=4,
                   dynamic_dma_scratch_size=SCRATCH)

    xT_d = nc.dram_tensor("xT", [128, SHP], f16, kind="ExternalInput")
    tab0_d = nc.dram_tensor("tab0", [TR, 2 * DIN], f8, kind="ExternalInput")
    disP_d = nc.dram_tensor("disP", [128, TILES], f32, kind="ExternalInput")
    disN_d = nc.dram_tensor("disN", [128, TILES], f32, kind="ExternalInput")
    dis2N_d = nc.dram_tensor("dis2N", [128, TILES], f32, kind="ExternalInput")
    W1_d = nc.dram_tensor("Wcat1", [DIN, 3 * HID], f16, kind="ExternalInput")
    W2_d = nc.dram_tensor("Wcat2", [HID, 3 * OUT], f16, kind="ExternalInput")
    AC1_d = nc.dram_tensor("AC1", [128, 2 * HID], f32, kind="ExternalInput")
    AC2_d = nc.dram_tensor("AC2", [128, 2 * OUT], f32, kind="ExternalInput")
    idx_d = nc.dram_tensor("idxs", [128, S_total // 16], i16, kind="ExternalInput")
    dloc_d = nc.dram_tensor("dloc", [128, B_total], f16, kind="ExternalInput")
    out_d = nc.dram_tensor("out", [SHP, OUT], f32, kind="ExternalOutput")

    rg = [list(range(N_CORES))]

    with tile.TileContext(nc) as tc:
        import contextlib
        ctx = contextlib.ExitStack()
        with ctx:
            const_p = ctx.enter_context(tc.tile_pool(name="const", bufs=1))
            big_p = ctx.enter_context(tc.tile_pool(name="big", bufs=1))
            g16_p = ctx.enter_context(tc.tile_pool(name="g16", bufs=26))
            s_p = ctx.enter_context(tc.tile_pool(name="sel", bufs=10))
            ev_p = ctx.enter_context(tc.tile_pool(name="ev", bufs=3))
            ps_prop = ctx.enter_context(tc.tile_pool(name="psprop", bufs=5, space="PSUM"))
            ps_dense = ctx.enter_context(tc.tile_pool(name="psdense", bufs=1, space="PSUM"))
            ps_tr = ctx.enter_context(tc.tile_pool(name="pstr", bufs=1, space="PSUM"))
            ps_ev = ctx.enter_context(tc.tile_pool(name="psev", bufs=1, space="PSUM"))
            dram_p = ctx.enter_context(tc.tile_pool(name="dram", bufs=1, space="DRAM"))

            # ---- constants ----
            iota16_sb = const_p.tile([128, 128], f16)
            nc.gpsimd.iota(iota16_sb[:], pattern=[[1, 128]], base=0,
                           channel_multiplier=0,
                           allow_small_or_imprecise_dtypes=True)
            ident = const_p.tile([128, 128], f16)
            make_identity(nc, ident[:])
            W1_sb = const_p.tile([DIN, 3 * HID], f16)
            nc.sync.dma_start(out=W1_sb[:], in_=W1_d.ap())
            W2_sb = const_p.tile([HID, 3 * OUT], f16)
            nc.sync.dma_start(out=W2_sb[:], in_=W2_d.ap())
            AC1_sb = const_p.tile([128, 2 * HID], f32)
            nc.sync.dma_start(out=AC1_sb[:], in_=AC1_d.ap())
            AC2_sb = const_p.tile([128, 2 * OUT], f32)
            nc.sync.dma_start(out=AC2_sb[:], in_=AC2_d.ap())
            disP_sb = const_p.tile([128, TILES], f32)
            nc.sync.dma_start(out=disP_sb[:], in_=disP_d.ap())
            disN_sb = const_p.tile([128, TILES], f32)
            nc.sync.dma_start(out=disN_sb[:], in_=disN_d.ap())
            dis2N_sb = const_p.tile([128, TILES], f32)
            nc.sync.dma_start(out=dis2N_sb[:], in_=dis2N_d.ap())
            dloc16_sb = const_p.tile([128, B_total], f16)
            nc.sync.dma_start(out=dloc16_sb[:], in_=dloc_d.ap())
            idx_sb = const_p.tile([128, S_total // 16], i16)
            nc.sync.dma_start(out=idx_sb[:], in_=idx_d.ap())

            # ---- big resident arrays ----
            xT_sb = big_p.tile([128, SHP], f16, tag="xT")
            nc.sync.dma_start(out=xT_sb[:], in_=xT_d.ap())

            gq = [0]  # rotating gather queue
            rep_i = [0]

            def run_body():
                ri = rep_i[0]
                rep_i[0] += 1
                # ---- DRAM bounce + tables (fresh per repeat; Shared AG outs) ----
                za_sb = big_p.tile([128, TILES * HID], f16, tag="za")
                zb_sb = big_p.tile([128, TILES * HID], f16, tag="zb")
                b2_t = dram_p.tile([SHP, 2 * HID], f8, tag=f"b2_{ri}")
                t2_t = dram_p.tile([TR, 2 * HID], f8, addr_space="Shared", tag=f"t2_{ri}")
                b3_t = dram_p.tile([SHP, HID], f16, tag=f"b3_{ri}")
                t3_t = dram_p.tile([TR, HID], f16, addr_space="Shared", tag=f"t3_{ri}")
                b4_t = dram_p.tile([SHP, HID], f16, tag=f"b4_{ri}")
                t4_t = dram_p.tile([TR, HID], f16, addr_space="Shared", tag=f"t4_{ri}")
                # gathering from the Shared-window AG outputs runs ~2x slower
                # per descriptor than from normal DRAM; bounce each table into
                # a local tile (chunk-wise so early-chunk gathers can start
                # while later chunks still copy)
                t2L = dram_p.tile([TR, 2 * HID], f8, tag=f"t2L_{ri}")
                t3L = dram_p.tile([TR, HID], f16, tag=f"t3L_{ri}")
                t4L = dram_p.tile([TR, HID], f16, tag=f"t4L_{ri}")

                def localize(dst, src):
                    for ch in range(NCH):
                        r0 = ch * CHUNK
                        r1 = min((ch + 1) * CHUNK, TR)
                        nc.sync.dma_start(out=dst[r0:r1, :], in_=src[r0:r1, :])

                def dense(lhs_sb, W_sb, F, za_dst, zb_dst, bounce, ev_dtype,
                          skip_c=False):
                    """z = lhs.T @ [Wa|Wb|Wc]; za kept, zb=dis*z_b kept, z_c=dis*z_c -> bounce."""
                    nj = 2 if skip_c else 3
                    for t in range(TILES):
                        lhsT = lhs_sb[:, t * 128:(t + 1) * 128]
                        ps = ps_dense.tile([128, 3 * F], f32)
                        for j in range(nj):
                            nc.tensor.matmul(ps[:, j * F:(j + 1) * F], lhsT,
                                             W_sb[:, j * F:(j + 1) * F],
                                             start=True, stop=True)
                        nc.vector.tensor_copy(za_dst[:, t * F:(t + 1) * F], ps[:, 0:F])
                        nc.vector.tensor_scalar(zb_dst[:, t * F:(t + 1) * F],
                                                ps[:, F:2 * F], disP_sb[:, t:t + 1],
                                                None, mybir.AluOpType.mult)
                        if skip_c:
                            continue
                        zc = ev_p.tile([128, 3 * OUT if F == OUT else F], ev_dtype, tag="zc")
                        nc.vector.tensor_scalar(zc[:, :F], ps[:, 2 * F:3 * F],
                                                disP_sb[:, t:t + 1], None,
                                                mybir.AluOpType.mult)
                        nc.sync.dma_start(out=bounce[t * 128:(t + 1) * 128, 0:F],
                                          in_=zc[:, :F])

                em_g = parts in ("all", "gather", "gs")
                em_s = parts in ("all", "gs", "nogather")
                em_m = parts in ("all", "nogather")
                plan = meta["plan"]
                batch_tile = meta["batch_tile"]
                first_gb = meta["first_gb"]
                last_gb = meta["last_gb"]

                def propagate(table, F, gdt, g_pool, evac, swap=False,
                              table_early=None, n_early=0):
                    """y[dst] = sum_e table[src_e]; evac(t, psum) consumes PSUM.

                    swap=True computes the TRANSPOSED aggregate psum = g.T @ S
                    ([feat, dst] instead of [dst, feat]) so the evac can feed it
                    straight into a follow-up matmul as lhsT. S is built in the
                    gather dtype so the matmul operands match."""
                    psums = {}
                    s_dt = gdt  # uniform-dtype matmuls; mixed f16xf8 MMs stall PE
                    for ci, (ch, slot0, nb_i, gb0) in enumerate(plan):
                        # while the localize copies drain, the first calls read
                        # the Shared AG output directly (slower but not idle)
                        tab = table_early if (table_early is not None
                                              and ci < n_early) else table
                        rows0 = ch * CHUNK
                        rows1 = min((ch + 1) * CHUNK, TR)
                        ni = nb_i * 128
                        col0 = slot0 // 16
                        g = g_pool.tile([128, NI_MAX_BATCHES, F], gdt, tag="g")
                        if em_g:
                            ebytes = F * (1 if gdt == f8 else 2)
                            if ebytes % 256 == 0:
                                nc.gpsimd.dma_gather(
                                    out_ap=g[:, :nb_i, :], in_ap=tab[rows0:rows1, :],
                                    idxs_ap=idx_sb[:, col0:col0 + ni // 16], num_idxs=ni,
                                    num_idxs_reg=ni, elem_size=F,
                                    queue_num=gq[0] % 4)
                            else:
                                # narrow rows (e.g. 64 f16 = 128B) at 256B pitch
                                dma_gather_raw(
                                    nc, g[:, :nb_i, :], tab[rows0:rows1, :],
                                    idx_sb[:, col0:col0 + ni // 16], ni, F,
                                    256, gq[0] % 4)
                            gq[0] += 1
                        if em_s and WIDE_S:
                            Sw = s_p.tile([128, NI_MAX_BATCHES, 128], s_dt, tag="S")
                            nc.vector.tensor_tensor(
                                out=Sw[:, :nb_i, :],
                                in0=iota16_sb[:].unsqueeze(1).broadcast_to([128, nb_i, 128]),
                                in1=dloc16_sb[:, gb0:gb0 + nb_i].unsqueeze(2).broadcast_to([128, nb_i, 128]),
                                op=mybir.AluOpType.is_equal)
                        for b in range(nb_i):
                            gb = gb0 + b
                            t = batch_tile[gb]
                            if em_m:
                                lhs = Sw[:, b, :]
                                if t not in psums:
                                    psums[t] = ps_prop.tile([128, F], f32, tag="pp", name=f"pp_{t}")
                                if swap:
                                    nc.tensor.matmul(psums[t][:], g[:, b, :], lhs,
                                                     start=(gb == first_gb[t]),
                                                     stop=(gb == last_gb[t]))
                                else:
                                    nc.tensor.matmul(psums[t][:], lhs, g[:, b, :],
                                                     start=(gb == first_gb[t]),
                                                     stop=(gb == last_gb[t]))
                                if gb == last_gb[t]:
                                    evac(t, psums.pop(t))

                if not em_m:
                    # timing-only: 4 propagates' gather/S traffic vs input table
                    for _ in range(4):
                        propagate(tab0_d.ap(), HID, f8, g16_p, None)
                    return

                # ================= layer 1 =================
                # prop1 gathers tab0 = fp8(dis*x) directly (already replicated
                # at staging): A(dis*x) @ 2W2 == A(dis*(x@2W2)), so the dense
                # transform moves AFTER the aggregation -> no dense dependency
                # and no AllGather before the first propagate. The swapped
                # aggregation leaves psum = p1'.T = [xfeat, dst], which feeds
                # the per-tile @2W2 matmul as lhsT after a psum->SBUF copy.
                dense(xT_sb, W1_sb, HID, za_sb, zb_sb, None, f8, skip_c=True)

                def evac_p1(t, psT):
                    cT = ev_p.tile([128, HID], f16, tag="cT")
                    nc.vector.tensor_copy(cT[:], psT[:])
                    ps2 = ps_ev.tile([128, HID], f32)
                    nc.tensor.matmul(ps2[:], cT[:], W1_sb[:, 2 * HID:3 * HID],
                                     start=True, stop=True)
                    tmp = ev_p.tile([128, HID], f16, tag="tmp16")
                    nc.vector.tensor_scalar(tmp[:], ps2[:], dis2N_sb[:, t:t + 1], None,
                                            mybir.AluOpType.mult)
                    v = ev_p.tile([128, HID], f8, tag="v8")
                    nc.vector.tensor_tensor(out=v[:], in0=tmp[:],
                                            in1=zb_sb[:, t * HID:(t + 1) * HID],
                                            op=mybir.AluOpType.add)
                    nc.sync.dma_start(out=b2_t[t * 128:(t + 1) * 128, 0:HID], in_=v[:])

                propagate(tab0_d.ap(), HID, f8, g16_p, evac_p1, swap=True)

                if single:
                    nc.sync.dma_start(out=t2_t[0:SHP, :], in_=b2_t[:, :])
                else:
                    nc.gpsimd.collective_compute(
                        "AllGather", mybir.AluOpType.bypass,
                        ins=[b2_t[:, :]], outs=[t2_t[:, :]], replica_groups=rg)
                localize(t2L, t2_t)

                hT_sb = big_p.tile([128, SHP], f16, tag=("xT" if repeat == 1 else "hT"))  # reuse xT slot

                def evac_p2(t, ps):
                    s1 = ev_p.tile([128, HID], f32, tag="s1")
                    nc.vector.tensor_scalar(s1[:], ps[:], disN_sb[:, t:t + 1], None,
                                            mybir.AluOpType.mult)
                    s2 = ev_p.tile([128, HID], f32, tag="s2")
                    nc.vector.tensor_tensor(out=s2[:], in0=s1[:],
                                            in1=za_sb[:, t * HID:(t + 1) * HID],
                                            op=mybir.AluOpType.add)
                    s4 = ev_p.tile([128, HID], f32, tag="s2")
                    nc.vector.tensor_tensor(out=s4[:], in0=s2[:], in1=AC1_sb[:, HID:],
                                            op=mybir.AluOpType.add)
                    h = ev_p.tile([128, HID], f16, tag="h")
                    nc.vector.tensor_scalar(h[:], s4[:], 0.0, None,
                                            mybir.AluOpType.max)
                    pst = ps_tr.tile([128, 128], f16)
                    nc.tensor.transpose(out=pst[:], in_=h[:], identity=ident[:])
                    nc.vector.tensor_copy(hT_sb[:, t * 128:(t + 1) * 128], pst[:])

                propagate(t2L, HID, f8, g16_p, evac_p2)

                # ================= layer 2 =================
                za2_sb = big_p.tile([128, TILES * OUT], f32, tag="za")
                zb2_sb = big_p.tile([128, TILES * OUT], f32, tag="zb")

                def dense2():
                    for t in range(TILES):
                        lhsT = hT_sb[:, t * 128:(t + 1) * 128]
                        ps = ps_dense.tile([128, 3 * OUT], f32)
                        for j in range(3):
                            nc.tensor.matmul(ps[:, j * OUT:(j + 1) * OUT], lhsT,
                                             W2_sb[:, j * OUT:(j + 1) * OUT],
                                             start=True, stop=True)
                        nc.vector.tensor_copy(za2_sb[:, t * OUT:(t + 1) * OUT], ps[:, 0:OUT])
                        nc.vector.tensor_scalar(zb2_sb[:, t * OUT:(t + 1) * OUT],
                                                ps[:, OUT:2 * OUT], disP_sb[:, t:t + 1],
                                                None, mybir.AluOpType.mult)
                        zc = ev_p.tile([128, HID], f16, tag="zc")
                        nc.vector.tensor_scalar(zc[:, :OUT], ps[:, 2 * OUT:3 * OUT],
                                                disP_sb[:, t:t + 1], None,
                                                mybir.AluOpType.mult)
                        nc.vector.memset(zc[:, OUT:], 0.0)
                        nc.sync.dma_start(out=b3_t[t * 128:(t + 1) * 128, :],
                                          in_=zc[:, :])

                dense2()

                if single:
                    nc.sync.dma_start(out=t3_t[0:SHP, :], in_=b3_t[:, :])
                else:
                    nc.gpsimd.collective_compute(
                        "AllGather", mybir.AluOpType.bypass,
                        ins=[b3_t[:, :]], outs=[t3_t[:, :]], replica_groups=rg)
                localize(t3L, t3_t)

                def evac_p3(t, ps):
                    tmp = ev_p.tile([128, OUT], f32, tag="tmp32")
                    nc.vector.tensor_scalar(tmp[:], ps[:], dis2N_sb[:, t:t + 1], None,
                                            mybir.AluOpType.mult)
                    v = ev_p.tile([128, HID], f16, tag="v16b")
                    nc.vector.tensor_tensor(out=v[:, :OUT], in0=tmp[:],
                                            in1=zb2_sb[:, t * OUT:(t + 1) * OUT],
                                            op=mybir.AluOpType.add)
                    nc.vector.memset(v[:, OUT:], 0.0)
                    nc.sync.dma_start(out=b4_t[t * 128:(t + 1) * 128, :], in_=v[:, :])

                propagate(t3L, OUT, f16, g16_p, evac_p3)

                if single:
                    nc.sync.dma_start(out=t4_t[0:SHP, :], in_=b4_t[:, :])
                else:
                    nc.gpsimd.collective_compute(
                        "AllGather", mybir.AluOpType.bypass,
                        ins=[b4_t[:, :]], outs=[t4_t[:, :]], replica_groups=rg)
                localize(t4L, t4_t)

                def evac_p4(t, ps):
                    o1 = ev_p.tile([128, OUT], f32, tag="o1")
                    nc.vector.tensor_scalar(o1[:], ps[:], disN_sb[:, t:t + 1], None,
                                            mybir.AluOpType.mult)
                    o2 = ev_p.tile([128, OUT], f32, tag="o2")
                    nc.vector.tensor_tensor(out=o2[:], in0=o1[:],
                                            in1=za2_sb[:, t * OUT:(t + 1) * OUT],
                                            op=mybir.AluOpType.add)
                    o4 = ev_p.tile([128, OUT], f32, tag="o2")
                    nc.vector.tensor_tensor(out=o4[:], in0=o2[:], in1=AC2_sb[:, OUT:],
                                            op=mybir.AluOpType.add)
                    nc.sync.dma_start(out=out_d.ap()[t * 128:(t + 1) * 128, :], in_=o4[:])

                propagate(t4L, OUT, f16, g16_p, evac_p4)

            for _rep in range(repeat):
                run_body()


    if do_compile:
        nc.compile()
    return nc


# ---------------------------------------------------------------------------
# SPMD runner (axon / PJRT path), kept warm across calls
# ---------------------------------------------------------------------------


class SpmdRunner:
    def __init__(self, nc, n_cores=N_CORES):
        import jax
        from jax.sharding import Mesh, PartitionSpec, NamedSharding
        from jax.experimental.shard_map import shard_map
        from concourse.bass2jax import (_bass_exec_p, partition_id_tensor,
                                        install_neuronx_cc_hook)
        install_neuronx_cc_hook()
        self.jax = jax
        self.n_cores = n_cores
        partition_name = nc.partition_id_tensor.name if nc.partition_id_tensor else None
        in_names, out_names, out_avals, zero_outs = [], [], [], []
        for alloc in nc.m.functions[0].allocations:
            if not isinstance(alloc, mybir.MemoryLocationSet):
                continue
            name = alloc.memorylocations[0].name
            if alloc.kind == "ExternalInput":
                if name != partition_name:
                    in_names.append(name)
            elif alloc.kind == "ExternalOutput":
                out_names.append(name)
                shape = tuple(alloc.tensor_shape)
                dtype = mybir.dt.np(alloc.dtype)
                out_avals.append(jax.core.ShapedArray(shape, dtype))
                zero_outs.append(np.zeros(shape, dtype))
        self.in_names, self.out_names = in_names, out_names
        self.out_avals, self.zero_outs = out_avals, zero_outs
        all_in_names = list(in_names) + list(out_names)
        if partition_name is not None:
            all_in_names.append(partition_name)

        def _body(*args):
            operands = list(args)
            if partition_name is not None:
                operands.append(partition_id_tensor())
            outs = _bass_exec_p.bind(
                *operands,
                out_avals=tuple(out_avals),
                in_names=tuple(all_in_names),
                out_names=tuple(out_names),
                lowering_input_output_aliases=(),
                sim_require_finite=True,
                sim_require_nnan=True,
                nc=nc,
            )
            return tuple(outs)

        devices = jax.devices()[:n_cores]
        self.mesh = Mesh(np.asarray(devices), ("core",))
        spec = PartitionSpec("core")
        self.sharding = NamedSharding(self.mesh, spec)
        in_specs = (spec,) * (len(in_names) + len(out_names))
        out_specs = (spec,) * len(out_names)
        self.fn = jax.jit(
            shard_map(_body, mesh=self.mesh, in_specs=in_specs,
                      out_specs=out_specs, check_rep=False),
            keep_unused=True,
        )

    def stage(self, in_maps):
        concat_in = [
            np.concatenate([np.asarray(in_maps[c][n]) for c in range(self.n_cores)], axis=0)
            for n in self.in_names
        ]
        concat_zeros = [
            np.zeros((self.n_cores * z.shape[0], *z.shape[1:]), z.dtype)
            for z in self.zero_outs
        ]
        dev = [self.jax.device_put(a, self.sharding) for a in concat_in + concat_zeros]
        self.jax.block_until_ready(dev)
        return dev

    def run(self, staged):
        out = self.fn(*staged)
        self.jax.block_until_ready(out)
        return out

    def unpack(self, out_arrs):
        res = []
        for c in range(self.n_cores):
            d = {}
            for i, n in enumerate(self.out_names):
                d[n] = np.asarray(out_arrs[i]).reshape(
                    self.n_cores, *self.out_avals[i].shape)[c]
            res.append(d)
        return res


_CACHE = {}


def _get_runner(cfg, meta):
    key = (tuple(sorted(cfg.items())), meta["nb_shared"].tobytes())
    if key not in _CACHE:
        nc = build_program(cfg, meta)
        _CACHE[key] = SpmdRunner(nc)
    return _CACHE[key]


def run_model(x, edge_index, weights, cfg):
    meta, dis, idx_w, dloc_t = preprocess_edges(edge_index, cfg)
    in_maps = build_host_inputs(x, dis, weights, cfg)
    for c in range(N_CORES):
        in_maps[c]["idxs"] = idx_w[c]
        in_maps[c]["dloc"] = dloc_t[c]
    r = _get_runner(cfg, meta)
    staged = r.stage(in_maps)
    res = r.unpack(r.run(staged))
    N, SH, OUT = cfg["N"], cfg["SH"], cfg["OUT"]
    out = np.empty((N, OUT), np.float32)
    for c in range(N_CORES):
        out[c * SH:(c + 1) * SH] = res[c]["out"][:SH]
    return out


def kernel(x, edge_index, W1, b1, W2, b2, g1, beta1, m1, v1, g2, beta2, m2, v2):
    x = np.asarray(x, np.float32)
    edge_index = np.asarray(edge_index)
    weights = tuple(np.asarray(w, np.float32) for w in
                    (W1, b1, W2, b2, g1, beta1, m1, v1, g2, beta2, m2, v2))
    return run_model(x, edge_index, weights, CFG)

